# revision 10
# baseline (speedup 1.0000x reference)
"""Trainium2 Bass kernel for nn_MemoryBankV2 (memory-bank attention block).

Strategy: the memory bank is the *original* (detached) input features, so
batch items are fully independent -> shard batch B=128 contiguously across
8 NeuronCores (16 items / core), replicate the memory bank; zero collectives.

On-chip compute per core, all activations in "Col" layout [feature-on-
partitions, rows-in-free]:
  - kT/v/q projections as bf16 matmuls (fp32 PSUM accumulate)
  - scores computed transposed sT[m, r] = kT.T @ qT, exp on ScalarE without
    max-subtraction (scores are O(1) by construction), multiplicative
    visibility mask fused into one VectorE scalar_tensor_tensor op
  - softmax normalization deferred: attnT = v.T @ E, Z via ones-matmul,
    divide at the end (Z + 1e-9 keeps item-0 rows finite; item-0 output is
    replaced via a flag blend exactly like the reference's where())
  - LayerNorm over the partition axis via (1/D)*ones matmuls for mean and
    E[x^2]
  - final sigmoid gate; output written back in Col layout, host transposes.
"""

import os
import sys

import numpy as np

sys.path.insert(0, "/opt/trn_rl_repo")

import ml_dtypes  # noqa: E402

import concourse.bass as bass  # noqa: E402
import concourse.mybir as mybir  # noqa: E402
import concourse.tile as tile  # noqa: E402
from concourse import bacc  # noqa: E402
from concourse.bass import ds  # noqa: E402
from concourse.bass_utils import run_bass_kernel_spmd  # noqa: E402

B, T, D, L = 128, 32, 512, 2
NCORES = 8
BLOC = B // NCORES      # 16 items per core
R = BLOC * T            # 512 rows per core
M = B * T               # 4096 memory entries
DT = D // 128           # 4 feature subtiles
FT = (4 * D) // 128     # 16 ffn subtiles
MT = M // 128           # 32 memory subtiles
NCHUNK = M // 512       # 8 memory chunks (512 each)

F32 = mybir.dt.float32
BF16 = mybir.dt.bfloat16
AF = mybir.ActivationFunctionType
ALU = mybir.AluOpType
BF = ml_dtypes.bfloat16

# params tensor column layout (per layer l, base = 48*l)
#   bq: +0..3, bk: +4..7, g1: +12..15, be1: +16..19,
#   b1: +20..35, b2: +36..39, g2: +40..43, be2: +44..47
# globals: bs: 96..99
P_BQ, P_BK, P_G1, P_BE1, P_B1, P_B2, P_G2, P_BE2 = 0, 4, 12, 16, 20, 36, 40, 44
P_BS = 96
P_COLS = 100


def _layernorm(nc, psum, tmps, x, prm, gcol, bcol, out_bf, onesi, epsln):
    """In-place LN over the partition axis of x ([128, DT, R] fp32);
    also writes a bf16 copy to out_bf."""
    mups = psum.tile([128, R], F32, tag="mm", bufs=3, name="ln_mu")
    for a in range(DT):
        nc.tensor.matmul(mups, onesi, x[:, a, :], start=(a == 0), stop=(a == DT - 1))
    sqps = psum.tile([128, R], F32, tag="mm", bufs=3, name="ln_sq")
    for a in range(DT):
        sq = tmps.tile([128, R], F32, tag="sq", bufs=2, name="ln_sqt")
        nc.vector.tensor_mul(sq, x[:, a, :], x[:, a, :])
        nc.tensor.matmul(sqps, onesi, sq, start=(a == 0), stop=(a == DT - 1))
    mu = tmps.tile([128, R], F32, tag="lnmu", bufs=1, name="ln_mub")
    nc.vector.tensor_scalar(out=mu, in0=mups, scalar1=0.0, scalar2=None, op0=ALU.add)
    mu2 = tmps.tile([128, R], F32, tag="lns", bufs=2, name="ln_mu2")
    nc.vector.tensor_mul(mu2, mu, mu)
    var = tmps.tile([128, R], F32, tag="lns", bufs=2, name="ln_var")
    nc.vector.tensor_sub(var, sqps, mu2)
    sd = tmps.tile([128, R], F32, tag="lns", bufs=2, name="ln_sd")
    nc.scalar.activation(out=sd, in_=var, func=AF.Sqrt, bias=epsln, scale=1.0)
    rstd = tmps.tile([128, R], F32, tag="lns", bufs=2, name="ln_rstd")
    nc.vector.reciprocal(rstd, sd)
    # rg = rstd * gain (gain broadcast per-partition)
    rg = tmps.tile([128, R], F32, tag="lnrg", bufs=1, name="ln_rg")
    nc.vector.tensor_scalar(out=rg, in0=rstd, scalar1=prm[:, gcol:gcol + 1],
                            scalar2=None, op0=ALU.mult)
    for a in range(DT):
        nc.vector.tensor_sub(x[:, a, :], x[:, a, :], mu)
        t = tmps.tile([128, R], F32, tag="lnt", bufs=2, name="ln_t")
        nc.vector.tensor_mul(t, x[:, a, :], rg)
        nc.vector.tensor_scalar(out=x[:, a, :], in0=t,
                                scalar1=prm[:, bcol + a:bcol + a + 1],
                                scalar2=None, op0=ALU.add)
        nc.vector.tensor_scalar(out=out_bf[:, a, :], in0=t,
                                scalar1=prm[:, bcol + a:bcol + a + 1],
                                scalar2=None, op0=ALU.add)


def _build():
    nc = bacc.Bacc("TRN2", target_bir_lowering=False, debug=False)

    memT_d = nc.dram_tensor("memT", [D, M], BF16, kind="ExternalInput").ap()
    xT0_d = nc.dram_tensor("xT0", [D, R], F32, kind="ExternalInput").ap()
    xT0bf_d = nc.dram_tensor("xT0bf", [D, R], BF16, kind="ExternalInput").ap()
    bb_d = nc.dram_tensor("b_bcast", [128, R], BF16, kind="ExternalInput").ap()
    iv_d = nc.dram_tensor("item_vals", [128, MT], BF16, kind="ExternalInput").ap()
    fl_d = nc.dram_tensor("flag0", [128, R], F32, kind="ExternalInput").ap()
    prm_d = nc.dram_tensor("params", [128, P_COLS], F32, kind="ExternalInput").ap()
    wq_d, wk_d, wv_d, w1_d, w2_d, bvb_d = [], [], [], [], [], []
    for l in range(L):
        wq_d.append(nc.dram_tensor(f"wq{l}", [D, D], BF16, kind="ExternalInput").ap())
        wk_d.append(nc.dram_tensor(f"wk{l}", [D, D], BF16, kind="ExternalInput").ap())
        wv_d.append(nc.dram_tensor(f"wv{l}", [D, D], BF16, kind="ExternalInput").ap())
        w1_d.append(nc.dram_tensor(f"w1{l}", [D, 4 * D], BF16, kind="ExternalInput").ap())
        w2_d.append(nc.dram_tensor(f"w2{l}", [4 * D, D], BF16, kind="ExternalInput").ap())
        bvb_d.append(nc.dram_tensor(f"bvb{l}", [128, D], BF16, kind="ExternalInput").ap())
    ws_d = nc.dram_tensor("ws", [2 * D, D], BF16, kind="ExternalInput").ap()
    out_d = nc.dram_tensor("outT", [D, R], F32, kind="ExternalOutput").ap()

    with tile.TileContext(nc) as tc:
        with (
            tc.tile_pool(name="sb", bufs=1) as sb,
            tc.tile_pool(name="ps", bufs=1, space="PSUM") as ps,
        ):
            # --- resident inputs -------------------------------------------------
            memT = sb.tile([128, DT, M], BF16, tag="memT", name="memT_sb")
            for a in range(DT):
                nc.sync.dma_start(out=memT[:, a, :], in_=memT_d[a * 128:(a + 1) * 128, :])
            x = sb.tile([128, DT, R], F32, tag="x", name="x_sb")
            x0 = sb.tile([128, DT, R], F32, tag="x0", name="x0_sb")
            x0bf = sb.tile([128, DT, R], BF16, tag="x0bf", name="x0bf_sb")
            for a in range(DT):
                sl = slice(a * 128, (a + 1) * 128)
                nc.sync.dma_start(out=x[:, a, :], in_=xT0_d[sl, :])
                nc.sync.dma_start(out=x0[:, a, :], in_=xT0_d[sl, :])
                nc.sync.dma_start(out=x0bf[:, a, :], in_=xT0bf_d[sl, :])
            bb = sb.tile([128, R], BF16, tag="bb", name="bb_sb")
            nc.sync.dma_start(out=bb, in_=bb_d[:, :])
            iv = sb.tile([128, MT], BF16, tag="iv", name="iv_sb")
            nc.sync.dma_start(out=iv, in_=iv_d[:, :])
            fl = sb.tile([128, R], F32, tag="fl", name="fl_sb")
            nc.sync.dma_start(out=fl, in_=fl_d[:, :])
            prm = sb.tile([128, P_COLS], F32, tag="prm", name="prm_sb")
            nc.sync.dma_start(out=prm, in_=prm_d[:, :])
            onesb = sb.tile([128, 128], BF16, tag="onesb", name="onesb_sb")
            nc.vector.memset(onesb, 1.0)
            onesi = sb.tile([128, 128], F32, tag="onesi", name="onesi_sb")
            nc.vector.memset(onesi, 1.0 / D)
            epsln = sb.tile([128, 1], F32, tag="epsln", name="epsln_sb")
            nc.vector.memset(epsln, 1e-5)

            xbf = sb.tile([128, DT, R], BF16, tag="xbf", bufs=2, name="xbf_sb0")
            for a in range(DT):
                nc.sync.dma_start(out=xbf[:, a, :], in_=xT0bf_d[a * 128:(a + 1) * 128, :])

            for l in range(L):
                base = 48 * l
                # --- layer weights ---------------------------------------------
                wq = sb.tile([128, DT, D], BF16, tag="wq", bufs=1, name="wq_sb")
                wk = sb.tile([128, DT, D], BF16, tag="wk", bufs=1, name="wk_sb")
                wv = sb.tile([128, DT, D], BF16, tag="wv", bufs=1, name="wv_sb")
                for a in range(DT):
                    sl = slice(a * 128, (a + 1) * 128)
                    nc.sync.dma_start(out=wq[:, a, :], in_=wq_d[l][sl, :])
                    nc.sync.dma_start(out=wk[:, a, :], in_=wk_d[l][sl, :])
                    nc.sync.dma_start(out=wv[:, a, :], in_=wv_d[l][sl, :])
                bvb = sb.tile([128, D], BF16, tag="bvb", bufs=1, name="bvb_sb")
                nc.sync.dma_start(out=bvb, in_=bvb_d[l][:, :])

                # --- q projection ----------------------------------------------
                qbf = sb.tile([128, DT, R], BF16, tag="qbf", bufs=2, name="q_sb")
                for j in range(DT):
                    qps = ps.tile([128, R], F32, tag="mm", bufs=3, name="q_ps")
                    for a in range(DT):
                        nc.tensor.matmul(qps, wq[:, a, ds(j * 128, 128)], xbf[:, a, :],
                                         start=(a == 0), stop=(a == DT - 1))
                    nc.vector.tensor_scalar(out=qbf[:, j, :], in0=qps,
                                            scalar1=prm[:, base + P_BQ + j:base + P_BQ + j + 1],
                                            scalar2=None, op0=ALU.add)

                # --- kT projection ([dout, m] in bf16, resident) ----------------
                kT = sb.tile([128, DT, M], BF16, tag="kT", name="kT_sb")
                for c in range(NCHUNK):
                    for j in range(DT):
                        kps = ps.tile([128, 512], F32, tag="mm", bufs=3, name="k_ps")
                        for a in range(DT):
                            nc.tensor.matmul(kps, wk[:, a, ds(j * 128, 128)],
                                             memT[:, a, ds(c * 512, 512)],
                                             start=(a == 0), stop=(a == DT - 1))
                        nc.vector.tensor_scalar(out=kT[:, j, ds(c * 512, 512)], in0=kps,
                                                scalar1=prm[:, base + P_BK + j:base + P_BK + j + 1],
                                                scalar2=None, op0=ALU.add)

                # --- attention --------------------------------------------------
                attnps = []
                for j in range(DT):
                    apj = ps.tile([128, R], F32, tag=f"attn{j}", bufs=1, name=f"attn_ps{j}")
                    attnps.append(apj)
                zps = ps.tile([128, R], F32, tag="z", bufs=1, name="z_ps")
                for mt in range(MT):
                    # v for this m-subtile: [m(128), dout(512)] += bias via bvb
                    vps = ps.tile([128, D], F32, tag="mm", bufs=3, name="v_ps")
                    for a in range(DT):
                        nc.tensor.matmul(vps, memT[:, a, ds(mt * 128, 128)], wv[:, a, :],
                                         start=(a == 0), stop=(a == DT - 1))
                    vsb = sb.tile([128, D], BF16, tag="vsb", bufs=4, name="v_sb")
                    nc.vector.tensor_add(vsb, vps, bvb)
                    # scores sT[m(128), r(512)]
                    sps = ps.tile([128, R], F32, tag="mm", bufs=3, name="s_ps")
                    for a in range(DT):
                        nc.tensor.matmul(sps, kT[:, a, ds(mt * 128, 128)], qbf[:, a, :],
                                         start=(a == 0), stop=(a == DT - 1))
                    eraw = sb.tile([128, R], BF16, tag="eraw", bufs=2, name="eraw_sb")
                    nc.scalar.activation(out=eraw, in_=sps, func=AF.Exp)
                    e = sb.tile([128, R], BF16, tag="e", bufs=4, name="e_sb")
                    nc.vector.scalar_tensor_tensor(out=e, in0=bb,
                                                   scalar=iv[:, mt:mt + 1], in1=eraw,
                                                   op0=ALU.is_gt, op1=ALU.mult)
                    nc.tensor.matmul(zps, onesb, e, start=(mt == 0), stop=(mt == MT - 1),
                                     skip_group_check=True)
                    for j in range(DT):
                        nc.tensor.matmul(attnps[j], vsb[:, ds(j * 128, 128)], e,
                                         start=(mt == 0), stop=(mt == MT - 1),
                                         skip_group_check=True)

                # normalize + residual into x
                zt = sb.tile([128, R], F32, tag="at", bufs=2, name="zt_sb")
                nc.scalar.activation(out=zt, in_=zps, func=AF.Copy, bias=1e-9)
                rz = sb.tile([128, R], F32, tag="rz", bufs=1, name="rz_sb")
                nc.vector.reciprocal(rz, zt)
                for j in range(DT):
                    at = sb.tile([128, R], F32, tag="at", bufs=2, name="at_sb")
                    nc.vector.tensor_mul(at, attnps[j], rz)
                    nc.vector.tensor_add(x[:, j, :], x[:, j, :], at)

                # LN1 (in place), bf16 copy for ffn
                xlnbf = sb.tile([128, DT, R], BF16, tag="xbf", bufs=2, name="xlnbf_sb")
                _layernorm(nc, ps, sb, x, prm, base + P_G1, base + P_BE1, xlnbf,
                           onesi, epsln)

                # FFN1 -> FFN2 fused over the 4D dim: h streams through a small
                # pool; FFN2 psums (one per output subtile) accumulate over o
                # and reuse the attn psum banks (freed above).
                f2ps = []
                for j in range(DT):
                    fpj = ps.tile([128, R], F32, tag=f"attn{j}", bufs=1, name=f"f2_ps{j}")
                    f2ps.append(fpj)
                for o in range(FT):
                    fps = ps.tile([128, R], F32, tag="mm", bufs=3, name="f1_ps")
                    for a in range(DT):
                        w1t = sb.tile([128, 128], BF16, tag="w1s", bufs=6, name="w1t_sb")
                        nc.sync.dma_start(out=w1t,
                                          in_=w1_d[l][a * 128:(a + 1) * 128, ds(o * 128, 128)])
                        nc.tensor.matmul(fps, w1t, xlnbf[:, a, :],
                                         start=(a == 0), stop=(a == DT - 1))
                    h = sb.tile([128, R], BF16, tag="h", bufs=4, name="h_sb")
                    nc.scalar.activation(out=h, in_=fps, func=AF.Gelu,
                                         bias=prm[:, base + P_B1 + o:base + P_B1 + o + 1],
                                         scale=1.0)
                    for j in range(DT):
                        w2t = sb.tile([128, 128], BF16, tag="w2s", bufs=6, name="w2t_sb")
                        nc.sync.dma_start(out=w2t,
                                          in_=w2_d[l][o * 128:(o + 1) * 128, ds(j * 128, 128)])
                        nc.tensor.matmul(f2ps[j], w2t, h,
                                         start=(o == 0), stop=(o == FT - 1),
                                         skip_group_check=True)
                for j in range(DT):
                    nc.vector.scalar_tensor_tensor(out=x[:, j, :], in0=f2ps[j],
                                                   scalar=prm[:, base + P_B2 + j:base + P_B2 + j + 1],
                                                   in1=x[:, j, :],
                                                   op0=ALU.add, op1=ALU.add)

                # LN2 (in place) + bf16 copy for next layer / gate
                xbf = sb.tile([128, DT, R], BF16, tag="xbf", bufs=2, name="xbf_sb")
                _layernorm(nc, ps, sb, x, prm, base + P_G2, base + P_BE2, xbf,
                           onesi, epsln)

            # --- item-0 blend: ao = x + flag0*(x0 - x) -------------------------
            ao = sb.tile([128, DT, R], F32, tag="ao", name="ao_sb")
            aobf = sb.tile([128, DT, R], BF16, tag="aobf", name="aobf_sb")
            for j in range(DT):
                dt_ = sb.tile([128, R], F32, tag="gt", bufs=4, name="dt_sb")
                nc.vector.tensor_sub(dt_, x0[:, j, :], x[:, j, :])
                fd = sb.tile([128, R], F32, tag="gt", bufs=4, name="fd_sb")
                nc.vector.tensor_mul(fd, fl, dt_)
                nc.vector.tensor_add(ao[:, j, :], x[:, j, :], fd)
                nc.vector.tensor_copy(out=aobf[:, j, :], in_=ao[:, j, :])

            # --- gate: g = sigmoid(ws.T @ [x0; ao] + bs) -----------------------
            for j in range(DT):
                gps = ps.tile([128, R], F32, tag="mm", bufs=3, name="g_ps")
                for c in range(2 * DT):
                    wst = sb.tile([128, 128], BF16, tag="wss", bufs=6, name="wst_sb")
                    nc.sync.dma_start(out=wst,
                                      in_=ws_d[c * 128:(c + 1) * 128, ds(j * 128, 128)])
                    rhs = x0bf[:, c, :] if c < DT else aobf[:, c - DT, :]
                    nc.tensor.matmul(gps, wst, rhs, start=(c == 0), stop=(c == 2 * DT - 1))
                g = sb.tile([128, R], F32, tag="gt", bufs=4, name="g_sb")
                nc.scalar.activation(out=g, in_=gps, func=AF.Sigmoid,
                                     bias=prm[:, P_BS + j:P_BS + j + 1], scale=1.0)
                # out = ao + g*(x0 - ao)
                d2 = sb.tile([128, R], F32, tag="gt", bufs=4, name="d2_sb")
                nc.vector.tensor_sub(d2, x0[:, j, :], ao[:, j, :])
                m2 = sb.tile([128, R], F32, tag="gt", bufs=4, name="m2_sb")
                nc.vector.tensor_mul(m2, g, d2)
                ov = sb.tile([128, R], F32, tag="gt", bufs=4, name="ov_sb")
                nc.vector.tensor_add(ov, ao[:, j, :], m2)
                nc.sync.dma_start(out=out_d[j * 128:(j + 1) * 128, :], in_=ov)

    nc.compile()
    return nc


_NC = None


def _get_nc():
    global _NC
    if _NC is None:
        _NC = _build()
    return _NC


def _make_in_maps(inputs):
    cog = np.asarray(inputs["cognition_features"], np.float32)
    flat = cog.reshape(M, D)
    cogT = np.ascontiguousarray(flat.T)          # [D, M] fp32
    memT_bf = cogT.astype(BF)
    sc = 1.0 / np.sqrt(np.float32(D))

    common = {"memT": memT_bf}
    for l in range(L):
        common[f"wq{l}"] = np.ascontiguousarray(np.asarray(inputs["Wq"][l], np.float32).T * sc).astype(BF)
        common[f"wk{l}"] = np.ascontiguousarray(np.asarray(inputs["Wk"][l], np.float32).T).astype(BF)
        common[f"wv{l}"] = np.ascontiguousarray(np.asarray(inputs["Wv"][l], np.float32).T).astype(BF)
        common[f"w1{l}"] = np.ascontiguousarray(np.asarray(inputs["W1"][l], np.float32).T).astype(BF)
        common[f"w2{l}"] = np.ascontiguousarray(np.asarray(inputs["W2"][l], np.float32).T).astype(BF)
        common[f"bvb{l}"] = np.broadcast_to(
            np.asarray(inputs["bv"][l], np.float32), (128, D)).astype(BF).copy()
    common["ws"] = np.ascontiguousarray(np.asarray(inputs["Ws"], np.float32).T).astype(BF)

    prm = np.zeros((128, P_COLS), np.float32)

    def put(col, vec):
        v = np.asarray(vec, np.float32).reshape(-1, 128)
        for j in range(v.shape[0]):
            prm[:, col + j] = v[j]

    for l in range(L):
        base = 48 * l
        put(base + P_BQ, np.asarray(inputs["bq"][l], np.float32) * sc)
        put(base + P_BK, inputs["bk"][l])
        put(base + P_G1, inputs["ln1_g"][l])
        put(base + P_BE1, inputs["ln1_b"][l])
        put(base + P_B1, inputs["b1"][l])
        put(base + P_B2, inputs["b2"][l])
        put(base + P_G2, inputs["ln2_g"][l])
        put(base + P_BE2, inputs["ln2_b"][l])
    put(P_BS, inputs["bs"])
    common["params"] = prm

    item_of_m = (np.arange(M) // T).astype(np.float32)
    iv = np.ascontiguousarray(item_of_m.reshape(MT, 128).T).astype(BF)
    common["item_vals"] = iv

    in_maps = []
    for d in range(NCORES):
        rows = slice(d * R, (d + 1) * R)
        b_of_r = (np.arange(d * R, (d + 1) * R) // T).astype(np.float32)
        im = dict(common)
        im["xT0"] = np.ascontiguousarray(cogT[:, rows])
        im["xT0bf"] = np.ascontiguousarray(cogT[:, rows]).astype(BF)
        im["b_bcast"] = np.broadcast_to(b_of_r, (128, R)).astype(BF).copy()
        im["flag0"] = np.broadcast_to((b_of_r == 0).astype(np.float32), (128, R)).copy()
        in_maps.append(im)
    return in_maps


def _run(in_maps, trace=False):
    nc = _get_nc()
    return run_bass_kernel_spmd(nc, in_maps, list(range(NCORES)), trace=trace)


def kernel(**inputs):
    in_maps = _make_in_maps(inputs)
    res = _run(in_maps)
    outT = np.empty((M, D), np.float32)
    for d in range(NCORES):
        outT[d * R:(d + 1) * R, :] = res.results[d]["outT"].T
    return outT.reshape(B, T, D)


if __name__ == "__main__":
    _build()
    print("build ok")


# revision 17
# speedup vs baseline: 1.0642x; 1.0642x over previous
"""Trainium2 Bass kernel for nn_MemoryBankV2 (memory-bank attention block).

Strategy: the memory bank is the *original* (detached) input features, so
batch items are fully independent -> shard batch B=128 contiguously across
8 NeuronCores (16 items / core), replicate the memory bank; zero collectives.

On-chip compute per core, all activations in "Col" layout [feature-on-
partitions, rows-in-free]:
  - kT/v/q projections as bf16 matmuls (fp32 PSUM accumulate)
  - scores computed transposed sT[m, r] = kT.T @ qT, exp on ScalarE without
    max-subtraction (scores are O(1) by construction), multiplicative
    visibility mask fused into one VectorE scalar_tensor_tensor op
  - softmax normalization deferred: attnT = v.T @ E, Z via ones-matmul,
    divide at the end (Z + 1e-9 keeps item-0 rows finite; item-0 output is
    replaced via a flag blend exactly like the reference's where())
  - LayerNorm over the partition axis via (1/D)*ones matmuls for mean and
    E[x^2]
  - final sigmoid gate; output written back in Col layout, host transposes.
"""

import os
import sys

import numpy as np

sys.path.insert(0, "/opt/trn_rl_repo")

import ml_dtypes  # noqa: E402

import concourse.bass as bass  # noqa: E402
import concourse.mybir as mybir  # noqa: E402
import concourse.tile as tile  # noqa: E402
from concourse import bacc  # noqa: E402
from concourse.bass import ds  # noqa: E402
from concourse.bass_utils import run_bass_kernel_spmd  # noqa: E402

B, T, D, L = 128, 32, 512, 2
NCORES = 8
BLOC = B // NCORES      # 16 items per core
R = BLOC * T            # 512 rows per core
M = B * T               # 4096 memory entries
DT = D // 128           # 4 feature subtiles
FT = (4 * D) // 128     # 16 ffn subtiles
MT = M // 128           # 32 memory subtiles
NCHUNK = M // 512       # 8 memory chunks (512 each)

F32 = mybir.dt.float32
BF16 = mybir.dt.bfloat16
AF = mybir.ActivationFunctionType
ALU = mybir.AluOpType
BF = ml_dtypes.bfloat16

# params tensor column layout (per layer l, base = 48*l)
#   bq: +0..3, bk: +4..7, g1: +12..15, be1: +16..19,
#   b1: +20..35, b2: +36..39, g2: +40..43, be2: +44..47
# globals: bs: 96..99
P_BQ, P_BK, P_G1, P_BE1, P_B1, P_B2, P_G2, P_BE2 = 0, 4, 12, 16, 20, 36, 40, 44
P_BS = 96
P_COLS = 100


def _layernorm(nc, psum, tmps, x, prm, gcol, bcol, out_bf, onesi, epsln,
               drain_filler=lambda n: None):
    """In-place LN over the partition axis of x ([128, DT, R] fp32);
    also writes a bf16 copy to out_bf."""
    mups = psum.tile([128, R], F32, tag="mm", bufs=3, name="ln_mu")
    for a in range(DT):
        nc.tensor.matmul(mups, onesi, x[:, a, :], start=(a == 0), stop=(a == DT - 1))
    sqps = psum.tile([128, R], F32, tag="mm", bufs=3, name="ln_sq")
    for a in range(DT):
        sq = tmps.tile([128, R], F32, tag="sq", bufs=2, name="ln_sqt")
        nc.vector.tensor_mul(sq, x[:, a, :], x[:, a, :])
        nc.tensor.matmul(sqps, onesi, sq, start=(a == 0), stop=(a == DT - 1))
    drain_filler(4)
    mu = tmps.tile([128, R], F32, tag="lnmu", bufs=1, name="ln_mub")
    nc.vector.tensor_scalar(out=mu, in0=mups, scalar1=0.0, scalar2=None, op0=ALU.add)
    mu2 = tmps.tile([128, R], F32, tag="lns", bufs=2, name="ln_mu2")
    nc.vector.tensor_mul(mu2, mu, mu)
    var = tmps.tile([128, R], F32, tag="lns", bufs=2, name="ln_var")
    nc.vector.tensor_sub(var, sqps, mu2)
    sd = tmps.tile([128, R], F32, tag="lns", bufs=2, name="ln_sd")
    nc.scalar.activation(out=sd, in_=var, func=AF.Sqrt, bias=epsln, scale=1.0)
    rstd = tmps.tile([128, R], F32, tag="lns", bufs=2, name="ln_rstd")
    nc.vector.reciprocal(rstd, sd)
    # rg = rstd * gain (gain broadcast per-partition)
    rg = tmps.tile([128, R], F32, tag="lnrg", bufs=1, name="ln_rg")
    nc.vector.tensor_scalar(out=rg, in0=rstd, scalar1=prm[:, gcol:gcol + 1],
                            scalar2=None, op0=ALU.mult)
    for a in range(DT):
        nc.vector.tensor_sub(x[:, a, :], x[:, a, :], mu)
        t = tmps.tile([128, R], F32, tag="lnt", bufs=2, name="ln_t")
        nc.vector.tensor_mul(t, x[:, a, :], rg)
        nc.vector.tensor_scalar(out=x[:, a, :], in0=t,
                                scalar1=prm[:, bcol + a:bcol + a + 1],
                                scalar2=None, op0=ALU.add)
        nc.vector.tensor_scalar(out=out_bf[:, a, :], in0=t,
                                scalar1=prm[:, bcol + a:bcol + a + 1],
                                scalar2=None, op0=ALU.add)


def _build():
    nc = bacc.Bacc("TRN2", target_bir_lowering=False, debug=False)

    memT_d = nc.dram_tensor("memT", [D, M], BF16, kind="ExternalInput").ap()
    xT0_d = nc.dram_tensor("xT0", [D, R], F32, kind="ExternalInput").ap()
    xT0bf_d = nc.dram_tensor("xT0bf", [D, R], BF16, kind="ExternalInput").ap()
    bb_d = nc.dram_tensor("b_bcast", [128, R], BF16, kind="ExternalInput").ap()
    iv_d = nc.dram_tensor("item_vals", [128, MT], BF16, kind="ExternalInput").ap()
    fl_d = nc.dram_tensor("flag0", [128, R], F32, kind="ExternalInput").ap()
    prm_d = nc.dram_tensor("params", [128, P_COLS], F32, kind="ExternalInput").ap()
    wq_d, wk_d, wv_d, w1_d, w2_d, bvb_d = [], [], [], [], [], []
    for l in range(L):
        wq_d.append(nc.dram_tensor(f"wq{l}", [D, D], BF16, kind="ExternalInput").ap())
        wk_d.append(nc.dram_tensor(f"wk{l}", [D, D], BF16, kind="ExternalInput").ap())
        wv_d.append(nc.dram_tensor(f"wv{l}", [D, D], BF16, kind="ExternalInput").ap())
        w1_d.append(nc.dram_tensor(f"w1{l}", [D, 4 * D], BF16, kind="ExternalInput").ap())
        w2_d.append(nc.dram_tensor(f"w2{l}", [4 * D, D], BF16, kind="ExternalInput").ap())
        bvb_d.append(nc.dram_tensor(f"bvb{l}", [128, D], BF16, kind="ExternalInput").ap())
    ws_d = nc.dram_tensor("ws", [2 * D, D], BF16, kind="ExternalInput").ap()
    out_d = nc.dram_tensor("outT", [D, R], F32, kind="ExternalOutput").ap()

    with tile.TileContext(nc) as tc:
        with (
            tc.tile_pool(name="sb", bufs=1) as sb,
            tc.tile_pool(name="ps", bufs=1, space="PSUM") as ps,
        ):
            # --- resident inputs -------------------------------------------------
            memT = sb.tile([128, DT, M], BF16, tag="memT", name="memT_sb")
            for a in range(DT):
                nc.sync.dma_start(out=memT[:, a, :], in_=memT_d[a * 128:(a + 1) * 128, :])
            x = sb.tile([128, DT, R], F32, tag="x", name="x_sb")
            x0 = sb.tile([128, DT, R], F32, tag="x0", name="x0_sb")
            x0bf = sb.tile([128, DT, R], BF16, tag="x0bf", name="x0bf_sb")
            for a in range(DT):
                sl = slice(a * 128, (a + 1) * 128)
                nc.sync.dma_start(out=x[:, a, :], in_=xT0_d[sl, :])
                nc.sync.dma_start(out=x0[:, a, :], in_=xT0_d[sl, :])
                nc.sync.dma_start(out=x0bf[:, a, :], in_=xT0bf_d[sl, :])
            bb = sb.tile([128, R], BF16, tag="bb", name="bb_sb")
            nc.sync.dma_start(out=bb, in_=bb_d[:, :])
            iv = sb.tile([128, MT], BF16, tag="iv", name="iv_sb")
            nc.sync.dma_start(out=iv, in_=iv_d[:, :])
            fl = sb.tile([128, R], F32, tag="fl", name="fl_sb")
            nc.sync.dma_start(out=fl, in_=fl_d[:, :])
            prm = sb.tile([128, P_COLS], F32, tag="prm", name="prm_sb")
            nc.sync.dma_start(out=prm, in_=prm_d[:, :])
            onesb = sb.tile([128, 128], BF16, tag="onesb", name="onesb_sb")
            nc.vector.memset(onesb, 1.0)
            onesi = sb.tile([128, 128], F32, tag="onesi", name="onesi_sb")
            nc.vector.memset(onesi, 1.0 / D)
            epsln = sb.tile([128, 1], F32, tag="epsln", name="epsln_sb")
            nc.vector.memset(epsln, 1e-5)

            xbf = sb.tile([128, DT, R], BF16, tag="xbf", bufs=2, name="xbf_sb0")
            for a in range(DT):
                nc.sync.dma_start(out=xbf[:, a, :], in_=xT0bf_d[a * 128:(a + 1) * 128, :])

            # per-layer weight/bias loads (bufs=2 so layer l+1 prefetches)
            def load_layer_weights(l):
                wq = sb.tile([128, DT, D], BF16, tag="wq", bufs=2, name="wq_sb")
                wk = sb.tile([128, DT, D], BF16, tag="wk", bufs=2, name="wk_sb")
                wv = sb.tile([128, DT, D], BF16, tag="wv", bufs=2, name="wv_sb")
                for a in range(DT):
                    sl = slice(a * 128, (a + 1) * 128)
                    nc.sync.dma_start(out=wq[:, a, :], in_=wq_d[l][sl, :])
                    nc.sync.dma_start(out=wk[:, a, :], in_=wk_d[l][sl, :])
                    nc.sync.dma_start(out=wv[:, a, :], in_=wv_d[l][sl, :])
                bvb = sb.tile([128, D], BF16, tag="bvb", bufs=2, name="bvb_sb")
                nc.sync.dma_start(out=bvb, in_=bvb_d[l][:, :])
                return wq, wk, wv, bvb

            def emit_kproj_group(l, wk, kT, c, j, tag="mm", tagbufs=3):
                base = 48 * l
                kps = ps.tile([128, 512], F32, tag=tag, bufs=tagbufs, name="k_ps")
                for a in range(DT):
                    nc.tensor.matmul(kps, wk[:, a, ds(j * 128, 128)],
                                     memT[:, a, ds(c * 512, 512)],
                                     start=(a == 0), stop=(a == DT - 1))
                nc.vector.tensor_scalar(out=kT[:, j, ds(c * 512, 512)], in0=kps,
                                        scalar1=prm[:, base + P_BK + j:base + P_BK + j + 1],
                                        scalar2=None, op0=ALU.add)

            layer_w = [load_layer_weights(0)]
            # filler queue: pending kproj groups of the *next* layer, drained
            # into the PE-idle zones (LN chains / FFN) of the current layer
            filler: list = []

            def drain_filler(n):
                for _ in range(min(n, len(filler))):
                    filler.pop(0)()

            for l in range(L):
                base = 48 * l
                wq, wk, wv, bvb = layer_w[l]

                # --- q projection ----------------------------------------------
                qbf = sb.tile([128, DT, R], BF16, tag="qbf", bufs=2, name="q_sb")
                for j in range(DT):
                    qps = ps.tile([128, R], F32, tag="mm", bufs=3, name="q_ps")
                    for a in range(DT):
                        nc.tensor.matmul(qps, wq[:, a, ds(j * 128, 128)], xbf[:, a, :],
                                         start=(a == 0), stop=(a == DT - 1))
                    nc.vector.tensor_scalar(out=qbf[:, j, :], in0=qps,
                                            scalar1=prm[:, base + P_BQ + j:base + P_BQ + j + 1],
                                            scalar2=None, op0=ALU.add)

                # --- kT projection ([dout, m] in bf16, resident) ----------------
                if l == 0:
                    kT = sb.tile([128, DT, M], BF16, tag="kT", name="kT_sb")
                    for c in range(NCHUNK):
                        for j in range(DT):
                            emit_kproj_group(0, wk, kT, c, j)
                else:
                    kT = kT_next  # noqa: F821  (built by layer l-1's filler)
                    drain_filler(len(filler))  # any leftovers

                # --- attention --------------------------------------------------
                attnps = []
                for j in range(DT):
                    apj = ps.tile([128, R], F32, tag=f"attn{j}", bufs=1, name=f"attn_ps{j}")
                    attnps.append(apj)
                zps = ps.tile([128, R], F32, tag="z", bufs=1, name="z_ps")
                for mt in range(MT):
                    # v for this m-subtile: [m(128), dout(512)] += bias via bvb
                    vps = ps.tile([128, D], F32, tag="mm", bufs=3, name="v_ps")
                    for a in range(DT):
                        nc.tensor.matmul(vps, memT[:, a, ds(mt * 128, 128)], wv[:, a, :],
                                         start=(a == 0), stop=(a == DT - 1))
                    vsb = sb.tile([128, D], BF16, tag="vsb", bufs=4, name="v_sb")
                    nc.vector.tensor_add(vsb, vps, bvb)
                    # scores sT[m(128), r(512)]
                    sps = ps.tile([128, R], F32, tag="mm", bufs=3, name="s_ps")
                    for a in range(DT):
                        nc.tensor.matmul(sps, kT[:, a, ds(mt * 128, 128)], qbf[:, a, :],
                                         start=(a == 0), stop=(a == DT - 1))
                    eraw = sb.tile([128, R], BF16, tag="eraw", bufs=2, name="eraw_sb")
                    nc.scalar.activation(out=eraw, in_=sps, func=AF.Exp)
                    e = sb.tile([128, R], BF16, tag="e", bufs=4, name="e_sb")
                    nc.vector.scalar_tensor_tensor(out=e, in0=bb,
                                                   scalar=iv[:, mt:mt + 1], in1=eraw,
                                                   op0=ALU.is_gt, op1=ALU.mult)
                    nc.tensor.matmul(zps, onesb, e, start=(mt == 0), stop=(mt == MT - 1),
                                     skip_group_check=True)
                    for j in range(DT):
                        nc.tensor.matmul(attnps[j], vsb[:, ds(j * 128, 128)], e,
                                         start=(mt == 0), stop=(mt == MT - 1),
                                         skip_group_check=True)

                # enqueue next layer's kT projection as PE filler work for the
                # LN/FFN zones below (it depends only on memT + next weights)
                if l + 1 < L:
                    layer_w.append(load_layer_weights(l + 1))
                    kT_next = sb.tile([128, DT, M], BF16, tag="kT", name="kTn_sb")
                    wk_next = layer_w[l + 1][1]
                    for c in range(NCHUNK):
                        for j in range(DT):
                            # alternate psum tags so filler groups pipeline
                            # (z bank is idle outside the attention loop)
                            tg, tb = ("z", 1) if (c * DT + j) % 2 else ("mm", 3)
                            filler.append(
                                lambda ll=l + 1, c=c, j=j, wkn=wk_next,
                                ktn=kT_next, tg=tg, tb=tb:
                                emit_kproj_group(ll, wkn, ktn, c, j, tg, tb))

                # normalize + residual into x
                zt = sb.tile([128, R], F32, tag="at", bufs=2, name="zt_sb")
                nc.scalar.activation(out=zt, in_=zps, func=AF.Copy, bias=1e-9)
                rz = sb.tile([128, R], F32, tag="rz", bufs=1, name="rz_sb")
                nc.vector.reciprocal(rz, zt)
                for j in range(DT):
                    at = sb.tile([128, R], F32, tag="at", bufs=2, name="at_sb")
                    nc.vector.tensor_mul(at, attnps[j], rz)
                    nc.vector.tensor_add(x[:, j, :], x[:, j, :], at)
                drain_filler(6)

                # LN1 (in place), bf16 copy for ffn
                xlnbf = sb.tile([128, DT, R], BF16, tag="xbf", bufs=2, name="xlnbf_sb")
                _layernorm(nc, ps, sb, x, prm, base + P_G1, base + P_BE1, xlnbf,
                           onesi, epsln, drain_filler)

                # FFN1 -> FFN2 fused over the 4D dim: h streams through a small
                # pool; FFN2 psums (one per output subtile) accumulate over o
                # and reuse the attn psum banks (freed above).
                f2ps = []
                for j in range(DT):
                    fpj = ps.tile([128, R], F32, tag=f"attn{j}", bufs=1, name=f"f2_ps{j}")
                    f2ps.append(fpj)
                for o in range(FT):
                    fps = ps.tile([128, R], F32, tag="mm", bufs=3, name="f1_ps")
                    for a in range(DT):
                        w1t = sb.tile([128, 128], BF16, tag="w1s", bufs=12, name="w1t_sb")
                        nc.sync.dma_start(out=w1t,
                                          in_=w1_d[l][a * 128:(a + 1) * 128, ds(o * 128, 128)])
                        nc.tensor.matmul(fps, w1t, xlnbf[:, a, :],
                                         start=(a == 0), stop=(a == DT - 1))
                    h = sb.tile([128, R], BF16, tag="h", bufs=4, name="h_sb")
                    nc.scalar.activation(out=h, in_=fps, func=AF.Gelu,
                                         bias=prm[:, base + P_B1 + o:base + P_B1 + o + 1],
                                         scale=1.0)
                    for j in range(DT):
                        w2t = sb.tile([128, 128], BF16, tag="w2s", bufs=12, name="w2t_sb")
                        nc.sync.dma_start(out=w2t,
                                          in_=w2_d[l][o * 128:(o + 1) * 128, ds(j * 128, 128)])
                        nc.tensor.matmul(f2ps[j], w2t, h,
                                         start=(o == 0), stop=(o == FT - 1),
                                         skip_group_check=True)
                    drain_filler(1)
                for j in range(DT):
                    nc.vector.scalar_tensor_tensor(out=x[:, j, :], in0=f2ps[j],
                                                   scalar=prm[:, base + P_B2 + j:base + P_B2 + j + 1],
                                                   in1=x[:, j, :],
                                                   op0=ALU.add, op1=ALU.add)

                # LN2 (in place) + bf16 copy for next layer / gate
                xbf = sb.tile([128, DT, R], BF16, tag="xbf", bufs=2, name="xbf_sb")
                _layernorm(nc, ps, sb, x, prm, base + P_G2, base + P_BE2, xbf,
                           onesi, epsln, drain_filler)

            # --- item-0 blend: ao = x + flag0*(x0 - x) -------------------------
            ao = sb.tile([128, DT, R], F32, tag="ao", name="ao_sb")
            aobf = sb.tile([128, DT, R], BF16, tag="aobf", name="aobf_sb")
            for j in range(DT):
                dt_ = sb.tile([128, R], F32, tag="gt", bufs=4, name="dt_sb")
                nc.vector.tensor_sub(dt_, x0[:, j, :], x[:, j, :])
                fd = sb.tile([128, R], F32, tag="gt", bufs=4, name="fd_sb")
                nc.vector.tensor_mul(fd, fl, dt_)
                nc.vector.tensor_add(ao[:, j, :], x[:, j, :], fd)
                nc.vector.tensor_copy(out=aobf[:, j, :], in_=ao[:, j, :])

            # --- gate: g = sigmoid(ws.T @ [x0; ao] + bs) -----------------------
            for j in range(DT):
                gps = ps.tile([128, R], F32, tag="mm", bufs=3, name="g_ps")
                for c in range(2 * DT):
                    wst = sb.tile([128, 128], BF16, tag="wss", bufs=6, name="wst_sb")
                    nc.sync.dma_start(out=wst,
                                      in_=ws_d[c * 128:(c + 1) * 128, ds(j * 128, 128)])
                    rhs = x0bf[:, c, :] if c < DT else aobf[:, c - DT, :]
                    nc.tensor.matmul(gps, wst, rhs, start=(c == 0), stop=(c == 2 * DT - 1))
                g = sb.tile([128, R], F32, tag="gt", bufs=4, name="g_sb")
                nc.scalar.activation(out=g, in_=gps, func=AF.Sigmoid,
                                     bias=prm[:, P_BS + j:P_BS + j + 1], scale=1.0)
                # out = ao + g*(x0 - ao)
                d2 = sb.tile([128, R], F32, tag="gt", bufs=4, name="d2_sb")
                nc.vector.tensor_sub(d2, x0[:, j, :], ao[:, j, :])
                m2 = sb.tile([128, R], F32, tag="gt", bufs=4, name="m2_sb")
                nc.vector.tensor_mul(m2, g, d2)
                ov = sb.tile([128, R], F32, tag="gt", bufs=4, name="ov_sb")
                nc.vector.tensor_add(ov, ao[:, j, :], m2)
                nc.sync.dma_start(out=out_d[j * 128:(j + 1) * 128, :], in_=ov)

    nc.compile()
    return nc


_NC = None


def _get_nc():
    global _NC
    if _NC is None:
        _NC = _build()
    return _NC


def _make_in_maps(inputs):
    cog = np.asarray(inputs["cognition_features"], np.float32)
    flat = cog.reshape(M, D)
    cogT = np.ascontiguousarray(flat.T)          # [D, M] fp32
    memT_bf = cogT.astype(BF)
    sc = 1.0 / np.sqrt(np.float32(D))

    common = {"memT": memT_bf}
    for l in range(L):
        common[f"wq{l}"] = np.ascontiguousarray(np.asarray(inputs["Wq"][l], np.float32).T * sc).astype(BF)
        common[f"wk{l}"] = np.ascontiguousarray(np.asarray(inputs["Wk"][l], np.float32).T).astype(BF)
        common[f"wv{l}"] = np.ascontiguousarray(np.asarray(inputs["Wv"][l], np.float32).T).astype(BF)
        common[f"w1{l}"] = np.ascontiguousarray(np.asarray(inputs["W1"][l], np.float32).T).astype(BF)
        common[f"w2{l}"] = np.ascontiguousarray(np.asarray(inputs["W2"][l], np.float32).T).astype(BF)
        common[f"bvb{l}"] = np.broadcast_to(
            np.asarray(inputs["bv"][l], np.float32), (128, D)).astype(BF).copy()
    common["ws"] = np.ascontiguousarray(np.asarray(inputs["Ws"], np.float32).T).astype(BF)

    prm = np.zeros((128, P_COLS), np.float32)

    def put(col, vec):
        v = np.asarray(vec, np.float32).reshape(-1, 128)
        for j in range(v.shape[0]):
            prm[:, col + j] = v[j]

    for l in range(L):
        base = 48 * l
        put(base + P_BQ, np.asarray(inputs["bq"][l], np.float32) * sc)
        put(base + P_BK, inputs["bk"][l])
        put(base + P_G1, inputs["ln1_g"][l])
        put(base + P_BE1, inputs["ln1_b"][l])
        put(base + P_B1, inputs["b1"][l])
        put(base + P_B2, inputs["b2"][l])
        put(base + P_G2, inputs["ln2_g"][l])
        put(base + P_BE2, inputs["ln2_b"][l])
    put(P_BS, inputs["bs"])
    common["params"] = prm

    item_of_m = (np.arange(M) // T).astype(np.float32)
    iv = np.ascontiguousarray(item_of_m.reshape(MT, 128).T).astype(BF)
    common["item_vals"] = iv

    in_maps = []
    for d in range(NCORES):
        rows = slice(d * R, (d + 1) * R)
        b_of_r = (np.arange(d * R, (d + 1) * R) // T).astype(np.float32)
        im = dict(common)
        im["xT0"] = np.ascontiguousarray(cogT[:, rows])
        im["xT0bf"] = np.ascontiguousarray(cogT[:, rows]).astype(BF)
        im["b_bcast"] = np.broadcast_to(b_of_r, (128, R)).astype(BF).copy()
        im["flag0"] = np.broadcast_to((b_of_r == 0).astype(np.float32), (128, R)).copy()
        in_maps.append(im)
    return in_maps


def _run(in_maps, trace=False):
    nc = _get_nc()
    return run_bass_kernel_spmd(nc, in_maps, list(range(NCORES)), trace=trace)


def kernel(**inputs):
    in_maps = _make_in_maps(inputs)
    res = _run(in_maps)
    outT = np.empty((M, D), np.float32)
    for d in range(NCORES):
        outT[d * R:(d + 1) * R, :] = res.results[d]["outT"].T
    return outT.reshape(B, T, D)


if __name__ == "__main__":
    _build()
    print("build ok")


# revision 22
# speedup vs baseline: 1.0671x; 1.0027x over previous
"""Trainium2 Bass kernel for nn_MemoryBankV2 (memory-bank attention block).

Strategy: the memory bank is the *original* (detached) input features, so
batch items are fully independent -> shard batch B=128 contiguously across
8 NeuronCores (16 items / core), replicate the memory bank; zero collectives.

On-chip compute per core, all activations in "Col" layout [feature-on-
partitions, rows-in-free]:
  - kT/v/q projections as bf16 matmuls (fp32 PSUM accumulate)
  - scores computed transposed sT[m, r] = kT.T @ qT, exp on ScalarE without
    max-subtraction (scores are O(1) by construction), multiplicative
    visibility mask fused into one VectorE scalar_tensor_tensor op
  - softmax normalization deferred: attnT = v.T @ E, Z via ones-matmul,
    divide at the end (Z + 1e-9 keeps item-0 rows finite; item-0 output is
    replaced via a flag blend exactly like the reference's where())
  - LayerNorm over the partition axis via (1/D)*ones matmuls for mean and
    E[x^2]
  - final sigmoid gate; output written back in Col layout, host transposes.
"""

import os
import sys

import numpy as np

sys.path.insert(0, "/opt/trn_rl_repo")

import ml_dtypes  # noqa: E402

import concourse.bass as bass  # noqa: E402
import concourse.mybir as mybir  # noqa: E402
import concourse.tile as tile  # noqa: E402
from concourse import bacc  # noqa: E402
from concourse.bass import ds  # noqa: E402
from concourse.bass_utils import run_bass_kernel_spmd  # noqa: E402

B, T, D, L = 128, 32, 512, 2
NCORES = 8
BLOC = B // NCORES      # 16 items per core
R = BLOC * T            # 512 rows per core
M = B * T               # 4096 memory entries
DT = D // 128           # 4 feature subtiles
FT = (4 * D) // 128     # 16 ffn subtiles
MT = M // 128           # 32 memory subtiles
NCHUNK = M // 512       # 8 memory chunks (512 each)

F32 = mybir.dt.float32
BF16 = mybir.dt.bfloat16
AF = mybir.ActivationFunctionType
ALU = mybir.AluOpType
BF = ml_dtypes.bfloat16

# params tensor column layout (per layer l, base = 48*l)
#   bq: +0..3, bk: +4..7, g1: +12..15, be1: +16..19,
#   b1: +20..35, b2: +36..39, g2: +40..43, be2: +44..47
# globals: bs: 96..99
P_BQ, P_BK, P_G1, P_BE1, P_B1, P_B2, P_G2, P_BE2 = 0, 4, 12, 16, 20, 36, 40, 44
P_BS = 96
P_COLS = 100


def _layernorm(nc, psum, tmps, x, prm, gcol, bcol, out_bf, onesi, epsln,
               drain_filler=lambda n: None):
    """In-place LN over the partition axis of x ([128, DT, R] fp32);
    also writes a bf16 copy to out_bf."""
    mups = psum.tile([128, R], F32, tag="mm", bufs=3, name="ln_mu")
    for a in range(DT):
        nc.tensor.matmul(mups, onesi, x[:, a, :], start=(a == 0), stop=(a == DT - 1))
    sqps = psum.tile([128, R], F32, tag="mm", bufs=3, name="ln_sq")
    for a in range(DT):
        sq = tmps.tile([128, R], F32, tag="sq", bufs=2, name="ln_sqt")
        nc.vector.tensor_mul(sq, x[:, a, :], x[:, a, :])
        nc.tensor.matmul(sqps, onesi, sq, start=(a == 0), stop=(a == DT - 1))
    drain_filler(4)
    mu = tmps.tile([128, R], F32, tag="lnmu", bufs=1, name="ln_mub")
    nc.vector.tensor_scalar(out=mu, in0=mups, scalar1=0.0, scalar2=None, op0=ALU.add)
    mu2 = tmps.tile([128, R], F32, tag="lns", bufs=2, name="ln_mu2")
    nc.vector.tensor_mul(mu2, mu, mu)
    var = tmps.tile([128, R], F32, tag="lns", bufs=2, name="ln_var")
    nc.vector.tensor_sub(var, sqps, mu2)
    sd = tmps.tile([128, R], F32, tag="lns", bufs=2, name="ln_sd")
    nc.scalar.activation(out=sd, in_=var, func=AF.Sqrt, bias=epsln, scale=1.0)
    rstd = tmps.tile([128, R], F32, tag="lns", bufs=2, name="ln_rstd")
    nc.vector.reciprocal(rstd, sd)
    # rg = rstd * gain (gain broadcast per-partition)
    rg = tmps.tile([128, R], F32, tag="lnrg", bufs=1, name="ln_rg")
    nc.vector.tensor_scalar(out=rg, in0=rstd, scalar1=prm[:, gcol:gcol + 1],
                            scalar2=None, op0=ALU.mult)
    for a in range(DT):
        nc.vector.tensor_sub(x[:, a, :], x[:, a, :], mu)
        t = tmps.tile([128, R], F32, tag="lnt", bufs=2, name="ln_t")
        nc.vector.tensor_mul(t, x[:, a, :], rg)
        nc.vector.tensor_scalar(out=x[:, a, :], in0=t,
                                scalar1=prm[:, bcol + a:bcol + a + 1],
                                scalar2=None, op0=ALU.add)
        nc.vector.tensor_scalar(out=out_bf[:, a, :], in0=t,
                                scalar1=prm[:, bcol + a:bcol + a + 1],
                                scalar2=None, op0=ALU.add)


def _build():
    nc = bacc.Bacc("TRN2", target_bir_lowering=False, debug=False)

    memT_d = nc.dram_tensor("memT", [D, M], BF16, kind="ExternalInput").ap()
    xT0_d = nc.dram_tensor("xT0", [D, R], F32, kind="ExternalInput").ap()
    xT0bf_d = nc.dram_tensor("xT0bf", [D, R], BF16, kind="ExternalInput").ap()
    bb_d = nc.dram_tensor("b_bcast", [128, R], BF16, kind="ExternalInput").ap()
    iv_d = nc.dram_tensor("item_vals", [128, MT], BF16, kind="ExternalInput").ap()
    fl_d = nc.dram_tensor("flag0", [128, R], F32, kind="ExternalInput").ap()
    prm_d = nc.dram_tensor("params", [128, P_COLS], F32, kind="ExternalInput").ap()
    wq_d, wk_d, wv_d, w1_d, w2_d, bvb_d = [], [], [], [], [], []
    for l in range(L):
        wq_d.append(nc.dram_tensor(f"wq{l}", [D, D], BF16, kind="ExternalInput").ap())
        wk_d.append(nc.dram_tensor(f"wk{l}", [D, D], BF16, kind="ExternalInput").ap())
        wv_d.append(nc.dram_tensor(f"wv{l}", [D, D], BF16, kind="ExternalInput").ap())
        w1_d.append(nc.dram_tensor(f"w1{l}", [D, 4 * D], BF16, kind="ExternalInput").ap())
        w2_d.append(nc.dram_tensor(f"w2{l}", [4 * D, D], BF16, kind="ExternalInput").ap())
        bvb_d.append(nc.dram_tensor(f"bvb{l}", [128, D], BF16, kind="ExternalInput").ap())
    ws_d = nc.dram_tensor("ws", [2 * D, D], BF16, kind="ExternalInput").ap()
    out_d = nc.dram_tensor("outT", [D, R], F32, kind="ExternalOutput").ap()

    with tile.TileContext(nc) as tc:
        with (
            tc.tile_pool(name="sb", bufs=1) as sb,
            tc.tile_pool(name="ps", bufs=1, space="PSUM") as ps,
        ):
            # --- resident inputs -------------------------------------------------
            memT = sb.tile([128, DT, M], BF16, tag="memT", name="memT_sb")
            for a in range(DT):
                nc.sync.dma_start(out=memT[:, a, :], in_=memT_d[a * 128:(a + 1) * 128, :])
            x = sb.tile([128, DT, R], F32, tag="x", name="x_sb")
            x0 = sb.tile([128, DT, R], F32, tag="x0", name="x0_sb")
            x0bf = sb.tile([128, DT, R], BF16, tag="x0bf", name="x0bf_sb")
            for a in range(DT):
                sl = slice(a * 128, (a + 1) * 128)
                nc.sync.dma_start(out=x[:, a, :], in_=xT0_d[sl, :])
                nc.sync.dma_start(out=x0[:, a, :], in_=xT0_d[sl, :])
                nc.sync.dma_start(out=x0bf[:, a, :], in_=xT0bf_d[sl, :])
            bb = sb.tile([128, R], BF16, tag="bb", name="bb_sb")
            nc.sync.dma_start(out=bb, in_=bb_d[:, :])
            iv = sb.tile([128, MT], BF16, tag="iv", name="iv_sb")
            nc.sync.dma_start(out=iv, in_=iv_d[:, :])
            fl = sb.tile([128, R], F32, tag="fl", name="fl_sb")
            nc.sync.dma_start(out=fl, in_=fl_d[:, :])
            prm = sb.tile([128, P_COLS], F32, tag="prm", name="prm_sb")
            nc.sync.dma_start(out=prm, in_=prm_d[:, :])
            onesb = sb.tile([128, 128], BF16, tag="onesb", name="onesb_sb")
            nc.vector.memset(onesb, 1.0)
            onesi = sb.tile([128, 128], F32, tag="onesi", name="onesi_sb")
            nc.vector.memset(onesi, 1.0 / D)
            epsln = sb.tile([128, 1], F32, tag="epsln", name="epsln_sb")
            nc.vector.memset(epsln, 1e-5)

            xbf = sb.tile([128, DT, R], BF16, tag="xbf", bufs=2, name="xbf_sb0")
            for a in range(DT):
                nc.sync.dma_start(out=xbf[:, a, :], in_=xT0bf_d[a * 128:(a + 1) * 128, :])

            # per-layer weight/bias loads (bufs=2 so layer l+1 prefetches)
            def load_layer_weights(l):
                wq = sb.tile([128, DT, D], BF16, tag="wq", bufs=2, name="wq_sb")
                wk = sb.tile([128, DT, D], BF16, tag="wk", bufs=2, name="wk_sb")
                wv = sb.tile([128, DT, D], BF16, tag="wv", bufs=2, name="wv_sb")
                for a in range(DT):
                    sl = slice(a * 128, (a + 1) * 128)
                    nc.sync.dma_start(out=wq[:, a, :], in_=wq_d[l][sl, :])
                    nc.sync.dma_start(out=wk[:, a, :], in_=wk_d[l][sl, :])
                    nc.sync.dma_start(out=wv[:, a, :], in_=wv_d[l][sl, :])
                bvb = sb.tile([128, D], BF16, tag="bvb", bufs=2, name="bvb_sb")
                nc.sync.dma_start(out=bvb, in_=bvb_d[l][:, :])
                return wq, wk, wv, bvb

            def emit_kproj_group(l, wk, kT, c, j, tag="mm", tagbufs=3,
                                 cast_on_scalar=False):
                base = 48 * l
                kps = ps.tile([128, 512], F32, tag=tag, bufs=tagbufs, name="k_ps")
                for a in range(DT):
                    nc.tensor.matmul(kps, wk[:, a, ds(j * 128, 128)],
                                     memT[:, a, ds(c * 512, 512)],
                                     start=(a == 0), stop=(a == DT - 1))
                dst = kT[:, j, ds(c * 512, 512)]
                bias_ap = prm[:, base + P_BK + j:base + P_BK + j + 1]
                if cast_on_scalar:
                    # keep filler casts off the DVE (busy with the LN chain)
                    nc.scalar.activation(out=dst, in_=kps, func=AF.Identity,
                                         bias=bias_ap, scale=1.0)
                else:
                    nc.vector.tensor_scalar(out=dst, in0=kps, scalar1=bias_ap,
                                            scalar2=None, op0=ALU.add)

            layer_w = [load_layer_weights(0)]
            # filler queue: pending kproj groups of the *next* layer, drained
            # into the PE-idle zones (LN chains / FFN) of the current layer
            filler: list = []

            def drain_filler(n):
                for _ in range(min(n, len(filler))):
                    filler.pop(0)()

            for l in range(L):
                base = 48 * l
                wq, wk, wv, bvb = layer_w[l]

                # --- q projection ----------------------------------------------
                qbf = sb.tile([128, DT, R], BF16, tag="qbf", bufs=2, name="q_sb")
                for j in range(DT):
                    qps = ps.tile([128, R], F32, tag="mm", bufs=3, name="q_ps")
                    for a in range(DT):
                        nc.tensor.matmul(qps, wq[:, a, ds(j * 128, 128)], xbf[:, a, :],
                                         start=(a == 0), stop=(a == DT - 1))
                    nc.vector.tensor_scalar(out=qbf[:, j, :], in0=qps,
                                            scalar1=prm[:, base + P_BQ + j:base + P_BQ + j + 1],
                                            scalar2=None, op0=ALU.add)

                # --- kT projection ([dout, m] in bf16, resident) ----------------
                if l == 0:
                    kT = sb.tile([128, DT, M], BF16, tag="kT", name="kT_sb")
                    for c in range(NCHUNK):
                        for j in range(DT):
                            emit_kproj_group(0, wk, kT, c, j)
                else:
                    kT = kT_next  # noqa: F821  (built by layer l-1's filler)
                    drain_filler(len(filler))  # any leftovers

                # --- attention --------------------------------------------------
                attnps = []
                for j in range(DT):
                    apj = ps.tile([128, R], F32, tag=f"attn{j}", bufs=1, name=f"attn_ps{j}")
                    attnps.append(apj)
                zps = ps.tile([128, R], F32, tag="z", bufs=1, name="z_ps")
                for mt in range(MT):
                    # v for this m-subtile: [m(128), dout(512)] += bias via bvb
                    vps = ps.tile([128, D], F32, tag="mm", bufs=3, name="v_ps")
                    for a in range(DT):
                        nc.tensor.matmul(vps, memT[:, a, ds(mt * 128, 128)], wv[:, a, :],
                                         start=(a == 0), stop=(a == DT - 1))
                    vsb = sb.tile([128, D], BF16, tag="vsb", bufs=4, name="v_sb")
                    nc.vector.tensor_add(vsb, vps, bvb)
                    # scores sT[m(128), r(512)]
                    sps = ps.tile([128, R], F32, tag="mm", bufs=3, name="s_ps")
                    for a in range(DT):
                        nc.tensor.matmul(sps, kT[:, a, ds(mt * 128, 128)], qbf[:, a, :],
                                         start=(a == 0), stop=(a == DT - 1))
                    eraw = sb.tile([128, R], BF16, tag="eraw", bufs=2, name="eraw_sb")
                    nc.scalar.activation(out=eraw, in_=sps, func=AF.Exp)
                    e = sb.tile([128, R], BF16, tag="e", bufs=4, name="e_sb")
                    nc.vector.scalar_tensor_tensor(out=e, in0=bb,
                                                   scalar=iv[:, mt:mt + 1], in1=eraw,
                                                   op0=ALU.is_gt, op1=ALU.mult)
                    nc.tensor.matmul(zps, onesb, e, start=(mt == 0), stop=(mt == MT - 1),
                                     skip_group_check=True)
                    for j in range(DT):
                        nc.tensor.matmul(attnps[j], vsb[:, ds(j * 128, 128)], e,
                                         start=(mt == 0), stop=(mt == MT - 1),
                                         skip_group_check=True)

                # enqueue next layer's kT projection as PE filler work for the
                # LN/FFN zones below (it depends only on memT + next weights)
                if l + 1 < L:
                    layer_w.append(load_layer_weights(l + 1))
                    kT_next = sb.tile([128, DT, M], BF16, tag="kT", name="kTn_sb")
                    wk_next = layer_w[l + 1][1]
                    for c in range(NCHUNK):
                        for j in range(DT):
                            # alternate psum tags so filler groups pipeline
                            # (z bank is idle outside the attention loop)
                            tg, tb = ("z", 1) if (c * DT + j) % 2 else ("mm", 3)
                            filler.append(
                                lambda ll=l + 1, c=c, j=j, wkn=wk_next,
                                ktn=kT_next, tg=tg, tb=tb:
                                emit_kproj_group(ll, wkn, ktn, c, j, tg, tb,
                                                 cast_on_scalar=True))

                # normalize + residual into x
                zt = sb.tile([128, R], F32, tag="at", bufs=2, name="zt_sb")
                nc.scalar.activation(out=zt, in_=zps, func=AF.Copy, bias=1e-9)
                rz = sb.tile([128, R], F32, tag="rz", bufs=1, name="rz_sb")
                nc.vector.reciprocal(rz, zt)
                for j in range(DT):
                    at = sb.tile([128, R], F32, tag="at", bufs=2, name="at_sb")
                    nc.vector.tensor_mul(at, attnps[j], rz)
                    nc.vector.tensor_add(x[:, j, :], x[:, j, :], at)
                drain_filler(6)

                # LN1 (in place), bf16 copy for ffn
                xlnbf = sb.tile([128, DT, R], BF16, tag="xbf", bufs=2, name="xlnbf_sb")
                _layernorm(nc, ps, sb, x, prm, base + P_G1, base + P_BE1, xlnbf,
                           onesi, epsln, drain_filler)

                # FFN1 -> FFN2 fused over the 4D dim: h streams through a small
                # pool; FFN2 psums (one per output subtile) accumulate over o
                # and reuse the attn psum banks (freed above).
                f2ps = []
                for j in range(DT):
                    fpj = ps.tile([128, R], F32, tag=f"attn{j}", bufs=1, name=f"f2_ps{j}")
                    f2ps.append(fpj)

                def emit_f2(h, o):
                    for j in range(DT):
                        w2t = sb.tile([128, 128], BF16, tag="w2s", bufs=12, name="w2t_sb")
                        nc.sync.dma_start(out=w2t,
                                          in_=w2_d[l][o * 128:(o + 1) * 128, ds(j * 128, 128)])
                        nc.tensor.matmul(f2ps[j], w2t, h,
                                         start=(o == 0), stop=(o == FT - 1),
                                         skip_group_check=True)

                # software-pipelined by 2: f2ps matmuls for step o are emitted
                # after FFN1 matmuls for step o+2, hiding the gelu latency from
                # the in-order PE queue
                hq = []
                for o in range(FT):
                    fps = ps.tile([128, R], F32, tag="mm", bufs=3, name="f1_ps")
                    for a in range(DT):
                        w1t = sb.tile([128, 128], BF16, tag="w1s", bufs=12, name="w1t_sb")
                        nc.sync.dma_start(out=w1t,
                                          in_=w1_d[l][a * 128:(a + 1) * 128, ds(o * 128, 128)])
                        nc.tensor.matmul(fps, w1t, xlnbf[:, a, :],
                                         start=(a == 0), stop=(a == DT - 1))
                    h = sb.tile([128, R], BF16, tag="h", bufs=4, name="h_sb")
                    nc.scalar.activation(out=h, in_=fps, func=AF.Gelu,
                                         bias=prm[:, base + P_B1 + o:base + P_B1 + o + 1],
                                         scale=1.0)
                    hq.append((h, o))
                    if len(hq) > 2:
                        emit_f2(*hq.pop(0))
                    drain_filler(1)
                for h_o in hq:
                    emit_f2(*h_o)
                for j in range(DT):
                    nc.vector.scalar_tensor_tensor(out=x[:, j, :], in0=f2ps[j],
                                                   scalar=prm[:, base + P_B2 + j:base + P_B2 + j + 1],
                                                   in1=x[:, j, :],
                                                   op0=ALU.add, op1=ALU.add)

                # pre-start the gate's x0 half on the freed attn psum banks so
                # those matmuls run under the final LN's serial chain
                if l == L - 1:
                    gps_l = []
                    for j in range(DT):
                        gp = ps.tile([128, R], F32, tag=f"attn{j}", bufs=1,
                                     name=f"g_ps{j}")
                        for c in range(DT):
                            wst = sb.tile([128, 128], BF16, tag="wss", bufs=8,
                                          name="wst_sb")
                            nc.sync.dma_start(
                                out=wst,
                                in_=ws_d[c * 128:(c + 1) * 128, ds(j * 128, 128)])
                            nc.tensor.matmul(gp, wst, x0bf[:, c, :],
                                             start=(c == 0), stop=False,
                                             skip_group_check=True)
                        gps_l.append(gp)

                # LN2 (in place) + bf16 copy for next layer / gate
                xbf = sb.tile([128, DT, R], BF16, tag="xbf", bufs=2, name="xbf_sb")
                _layernorm(nc, ps, sb, x, prm, base + P_G2, base + P_BE2, xbf,
                           onesi, epsln, drain_filler)

            # --- item-0 blend: ao = x + flag0*(x0 - x) -------------------------
            ao = sb.tile([128, DT, R], F32, tag="ao", name="ao_sb")
            aobf = sb.tile([128, DT, R], BF16, tag="aobf", name="aobf_sb")
            for j in range(DT):
                dt_ = sb.tile([128, R], F32, tag="gt", bufs=4, name="dt_sb")
                nc.vector.tensor_sub(dt_, x0[:, j, :], x[:, j, :])
                fd = sb.tile([128, R], F32, tag="gt", bufs=4, name="fd_sb")
                nc.vector.tensor_mul(fd, fl, dt_)
                nc.vector.tensor_add(ao[:, j, :], x[:, j, :], fd)
                nc.vector.tensor_copy(out=aobf[:, j, :], in_=ao[:, j, :])

            # --- gate: g = sigmoid(ws.T @ [x0; ao] + bs) -----------------------
            # (x0 half already accumulated in gps_l; finish with the ao half)
            for j in range(DT):
                gps = gps_l[j]
                for c in range(DT, 2 * DT):
                    wst = sb.tile([128, 128], BF16, tag="wss", bufs=8, name="wst_sb")
                    nc.sync.dma_start(out=wst,
                                      in_=ws_d[c * 128:(c + 1) * 128, ds(j * 128, 128)])
                    nc.tensor.matmul(gps, wst, aobf[:, c - DT, :],
                                     start=False, stop=(c == 2 * DT - 1),
                                     skip_group_check=True)
                g = sb.tile([128, R], F32, tag="gt", bufs=4, name="g_sb")
                nc.scalar.activation(out=g, in_=gps, func=AF.Sigmoid,
                                     bias=prm[:, P_BS + j:P_BS + j + 1], scale=1.0)
                # out = ao + g*(x0 - ao)
                d2 = sb.tile([128, R], F32, tag="gt", bufs=4, name="d2_sb")
                nc.vector.tensor_sub(d2, x0[:, j, :], ao[:, j, :])
                m2 = sb.tile([128, R], F32, tag="gt", bufs=4, name="m2_sb")
                nc.vector.tensor_mul(m2, g, d2)
                ov = sb.tile([128, R], F32, tag="gt", bufs=4, name="ov_sb")
                nc.vector.tensor_add(ov, ao[:, j, :], m2)
                nc.sync.dma_start(out=out_d[j * 128:(j + 1) * 128, :], in_=ov)

    nc.compile()
    return nc


_NC = None


def _get_nc():
    global _NC
    if _NC is None:
        _NC = _build()
    return _NC


def _make_in_maps(inputs):
    cog = np.asarray(inputs["cognition_features"], np.float32)
    flat = cog.reshape(M, D)
    cogT = np.ascontiguousarray(flat.T)          # [D, M] fp32
    memT_bf = cogT.astype(BF)
    sc = 1.0 / np.sqrt(np.float32(D))

    common = {"memT": memT_bf}
    for l in range(L):
        common[f"wq{l}"] = np.ascontiguousarray(np.asarray(inputs["Wq"][l], np.float32).T * sc).astype(BF)
        common[f"wk{l}"] = np.ascontiguousarray(np.asarray(inputs["Wk"][l], np.float32).T).astype(BF)
        common[f"wv{l}"] = np.ascontiguousarray(np.asarray(inputs["Wv"][l], np.float32).T).astype(BF)
        common[f"w1{l}"] = np.ascontiguousarray(np.asarray(inputs["W1"][l], np.float32).T).astype(BF)
        common[f"w2{l}"] = np.ascontiguousarray(np.asarray(inputs["W2"][l], np.float32).T).astype(BF)
        common[f"bvb{l}"] = np.broadcast_to(
            np.asarray(inputs["bv"][l], np.float32), (128, D)).astype(BF).copy()
    common["ws"] = np.ascontiguousarray(np.asarray(inputs["Ws"], np.float32).T).astype(BF)

    prm = np.zeros((128, P_COLS), np.float32)

    def put(col, vec):
        v = np.asarray(vec, np.float32).reshape(-1, 128)
        for j in range(v.shape[0]):
            prm[:, col + j] = v[j]

    for l in range(L):
        base = 48 * l
        put(base + P_BQ, np.asarray(inputs["bq"][l], np.float32) * sc)
        put(base + P_BK, inputs["bk"][l])
        put(base + P_G1, inputs["ln1_g"][l])
        put(base + P_BE1, inputs["ln1_b"][l])
        put(base + P_B1, inputs["b1"][l])
        put(base + P_B2, inputs["b2"][l])
        put(base + P_G2, inputs["ln2_g"][l])
        put(base + P_BE2, inputs["ln2_b"][l])
    put(P_BS, inputs["bs"])
    common["params"] = prm

    item_of_m = (np.arange(M) // T).astype(np.float32)
    iv = np.ascontiguousarray(item_of_m.reshape(MT, 128).T).astype(BF)
    common["item_vals"] = iv

    in_maps = []
    for d in range(NCORES):
        rows = slice(d * R, (d + 1) * R)
        b_of_r = (np.arange(d * R, (d + 1) * R) // T).astype(np.float32)
        im = dict(common)
        im["xT0"] = np.ascontiguousarray(cogT[:, rows])
        im["xT0bf"] = np.ascontiguousarray(cogT[:, rows]).astype(BF)
        im["b_bcast"] = np.broadcast_to(b_of_r, (128, R)).astype(BF).copy()
        im["flag0"] = np.broadcast_to((b_of_r == 0).astype(np.float32), (128, R)).copy()
        in_maps.append(im)
    return in_maps


def _run(in_maps, trace=False):
    nc = _get_nc()
    return run_bass_kernel_spmd(nc, in_maps, list(range(NCORES)), trace=trace)


def kernel(**inputs):
    in_maps = _make_in_maps(inputs)
    res = _run(in_maps)
    outT = np.empty((M, D), np.float32)
    for d in range(NCORES):
        outT[d * R:(d + 1) * R, :] = res.results[d]["outT"].T
    return outT.reshape(B, T, D)


if __name__ == "__main__":
    _build()
    print("build ok")


# revision 29
# speedup vs baseline: 1.2705x; 1.1906x over previous
"""Trainium2 Bass kernel for nn_MemoryBankV2 (memory-bank attention block).

Strategy: the memory bank is the *original* (detached) input features, so
batch items are fully independent -> shard batch B=128 contiguously across
8 NeuronCores (16 items / core), replicate the memory bank; zero collectives.

On-chip compute per core, all activations in "Col" layout [feature-on-
partitions, rows-in-free]:
  - kT/v/q projections as bf16 matmuls (fp32 PSUM accumulate)
  - scores computed transposed sT[m, r] = kT.T @ qT, exp on ScalarE without
    max-subtraction (scores are O(1) by construction), multiplicative
    visibility mask fused into one VectorE scalar_tensor_tensor op
  - softmax normalization deferred: attnT = v.T @ E, Z via ones-matmul,
    divide at the end (Z + 1e-9 keeps item-0 rows finite; item-0 output is
    replaced via a flag blend exactly like the reference's where())
  - LayerNorm over the partition axis via (1/D)*ones matmuls for mean and
    E[x^2]
  - final sigmoid gate; output written back in Col layout, host transposes.
"""

import os
import sys

import numpy as np

sys.path.insert(0, "/opt/trn_rl_repo")

import ml_dtypes  # noqa: E402

import concourse.bass as bass  # noqa: E402
import concourse.mybir as mybir  # noqa: E402
import concourse.tile as tile  # noqa: E402
from concourse import bacc  # noqa: E402
from concourse.bass import ds  # noqa: E402
from concourse.bass_utils import run_bass_kernel_spmd  # noqa: E402

B, T, D, L = 128, 32, 512, 2
NCORES = 8
BLOC = B // NCORES      # 16 items per core
R = BLOC * T            # 512 rows per core
M = B * T               # 4096 memory entries
DT = D // 128           # 4 feature subtiles
FT = (4 * D) // 128     # 16 ffn subtiles
MT = M // 128           # 32 memory subtiles
NCHUNK = M // 512       # 8 memory chunks (512 each)

F32 = mybir.dt.float32
BF16 = mybir.dt.bfloat16
AF = mybir.ActivationFunctionType
ALU = mybir.AluOpType
BF = ml_dtypes.bfloat16

# params tensor column layout (per layer l, base = 48*l)
#   bq: +0..3, bk: +4..7, g1: +12..15, be1: +16..19,
#   b1: +20..35, b2: +36..39, g2: +40..43, be2: +44..47
# globals: bs: 96..99
P_BQ, P_BK, P_G1, P_BE1, P_B1, P_B2, P_G2, P_BE2 = 0, 4, 12, 16, 20, 36, 40, 44
P_BS = 96
P_COLS = 100


def _layernorm(nc, psum, tmps, x, prm, gcol, bcol, out_bf, onesi, epsln,
               drain_filler=lambda n: None):
    """In-place LN over the partition axis of x ([128, DT, R] fp32);
    also writes a bf16 copy to out_bf."""
    mups = psum.tile([128, R], F32, tag="mm", bufs=3, name="ln_mu")
    for a in range(DT):
        nc.tensor.matmul(mups, onesi, x[:, a, :], start=(a == 0), stop=(a == DT - 1))
    sqps = psum.tile([128, R], F32, tag="mm", bufs=3, name="ln_sq")
    for a in range(DT):
        sq = tmps.tile([128, R], F32, tag="sq", bufs=2, name="ln_sqt")
        nc.vector.tensor_mul(sq, x[:, a, :], x[:, a, :])
        nc.tensor.matmul(sqps, onesi, sq, start=(a == 0), stop=(a == DT - 1))
    drain_filler(4)
    mu = tmps.tile([128, R], F32, tag="lnmu", bufs=1, name="ln_mub")
    nc.vector.tensor_scalar(out=mu, in0=mups, scalar1=0.0, scalar2=None, op0=ALU.add)
    mu2 = tmps.tile([128, R], F32, tag="lns", bufs=2, name="ln_mu2")
    nc.vector.tensor_mul(mu2, mu, mu)
    var = tmps.tile([128, R], F32, tag="lns", bufs=2, name="ln_var")
    nc.vector.tensor_sub(var, sqps, mu2)
    sd = tmps.tile([128, R], F32, tag="lns", bufs=2, name="ln_sd")
    nc.scalar.activation(out=sd, in_=var, func=AF.Sqrt, bias=epsln, scale=1.0)
    rstd = tmps.tile([128, R], F32, tag="lns", bufs=2, name="ln_rstd")
    nc.vector.reciprocal(rstd, sd)
    # rg = rstd * gain (gain broadcast per-partition)
    rg = tmps.tile([128, R], F32, tag="lnrg", bufs=1, name="ln_rg")
    nc.vector.tensor_scalar(out=rg, in0=rstd, scalar1=prm[:, gcol:gcol + 1],
                            scalar2=None, op0=ALU.mult)
    for a in range(DT):
        nc.vector.tensor_sub(x[:, a, :], x[:, a, :], mu)
        t = tmps.tile([128, R], F32, tag="lnt", bufs=2, name="ln_t")
        nc.vector.tensor_mul(t, x[:, a, :], rg)
        nc.vector.tensor_scalar(out=x[:, a, :], in0=t,
                                scalar1=prm[:, bcol + a:bcol + a + 1],
                                scalar2=None, op0=ALU.add)
        nc.vector.tensor_scalar(out=out_bf[:, a, :], in0=t,
                                scalar1=prm[:, bcol + a:bcol + a + 1],
                                scalar2=None, op0=ALU.add)


def _build():
    nc = bacc.Bacc("TRN2", target_bir_lowering=False, debug=False)

    memT_d = nc.dram_tensor("memT", [D, M], BF16, kind="ExternalInput").ap()
    xT0_d = nc.dram_tensor("xT0", [D, R], F32, kind="ExternalInput").ap()
    xT0bf_d = nc.dram_tensor("xT0bf", [D, R], BF16, kind="ExternalInput").ap()
    bb_d = nc.dram_tensor("b_bcast", [128, R], BF16, kind="ExternalInput").ap()
    iv_d = nc.dram_tensor("item_vals", [128, MT], BF16, kind="ExternalInput").ap()
    fl_d = nc.dram_tensor("flag0", [128, R], F32, kind="ExternalInput").ap()
    prm_d = nc.dram_tensor("params", [128, P_COLS], F32, kind="ExternalInput").ap()
    wq_d, wk_d, wv_d, w1_d, w2_d, bvb_d = [], [], [], [], [], []
    for l in range(L):
        wq_d.append(nc.dram_tensor(f"wq{l}", [D, D], BF16, kind="ExternalInput").ap())
        wk_d.append(nc.dram_tensor(f"wk{l}", [D, D], BF16, kind="ExternalInput").ap())
        wv_d.append(nc.dram_tensor(f"wv{l}", [D, D], BF16, kind="ExternalInput").ap())
        w1_d.append(nc.dram_tensor(f"w1{l}", [D, 4 * D], BF16, kind="ExternalInput").ap())
        w2_d.append(nc.dram_tensor(f"w2{l}", [4 * D, D], BF16, kind="ExternalInput").ap())
        bvb_d.append(nc.dram_tensor(f"bvb{l}", [128, D], BF16, kind="ExternalInput").ap())
    ws_d = nc.dram_tensor("ws", [2 * D, D], BF16, kind="ExternalInput").ap()
    out_d = nc.dram_tensor("outT", [D, R], F32, kind="ExternalOutput").ap()

    with tile.TileContext(nc) as tc:
        with (
            tc.tile_pool(name="sb", bufs=1) as sb,
            tc.tile_pool(name="ps", bufs=1, space="PSUM") as ps,
        ):
            # --- resident inputs -------------------------------------------------
            memT = sb.tile([128, DT, M], BF16, tag="memT", name="memT_sb")
            for a in range(DT):
                nc.sync.dma_start(out=memT[:, a, :], in_=memT_d[a * 128:(a + 1) * 128, :])
            x = sb.tile([128, DT, R], F32, tag="x", name="x_sb")
            x0 = sb.tile([128, DT, R], F32, tag="x0", name="x0_sb")
            x0bf = sb.tile([128, DT, R], BF16, tag="x0bf", name="x0bf_sb")
            for a in range(DT):
                sl = slice(a * 128, (a + 1) * 128)
                nc.sync.dma_start(out=x[:, a, :], in_=xT0_d[sl, :])
                nc.sync.dma_start(out=x0[:, a, :], in_=xT0_d[sl, :])
                nc.sync.dma_start(out=x0bf[:, a, :], in_=xT0bf_d[sl, :])
            bb = sb.tile([128, R], BF16, tag="bb", name="bb_sb")
            nc.sync.dma_start(out=bb, in_=bb_d[:, :])
            iv = sb.tile([128, MT], BF16, tag="iv", name="iv_sb")
            nc.sync.dma_start(out=iv, in_=iv_d[:, :])
            fl = sb.tile([128, R], F32, tag="fl", name="fl_sb")
            nc.sync.dma_start(out=fl, in_=fl_d[:, :])
            prm = sb.tile([128, P_COLS], F32, tag="prm", name="prm_sb")
            nc.sync.dma_start(out=prm, in_=prm_d[:, :])
            onesb = sb.tile([128, 128], BF16, tag="onesb", name="onesb_sb")
            nc.vector.memset(onesb, 1.0)
            onesi = sb.tile([128, 128], F32, tag="onesi", name="onesi_sb")
            nc.vector.memset(onesi, 1.0 / D)
            epsln = sb.tile([128, 1], F32, tag="epsln", name="epsln_sb")
            nc.vector.memset(epsln, 1e-5)

            xbf = sb.tile([128, DT, R], BF16, tag="xbf", bufs=2, name="xbf_sb0")
            for a in range(DT):
                nc.sync.dma_start(out=xbf[:, a, :], in_=xT0bf_d[a * 128:(a + 1) * 128, :])

            # per-layer weight/bias loads (bufs=2 so layer l+1 prefetches)
            def load_layer_weights(l):
                wq = sb.tile([128, DT, D], BF16, tag="wq", bufs=2, name="wq_sb")
                wk = sb.tile([128, DT, D], BF16, tag="wk", bufs=2, name="wk_sb")
                wv = sb.tile([128, DT, D], BF16, tag="wv", bufs=2, name="wv_sb")
                nc.sync.dma_start(out=wq, in_=wq_d[l].rearrange("(a p) n -> p a n", p=128))
                nc.sync.dma_start(out=wk, in_=wk_d[l].rearrange("(a p) n -> p a n", p=128))
                nc.sync.dma_start(out=wv, in_=wv_d[l].rearrange("(a p) n -> p a n", p=128))
                bvb = sb.tile([128, D], BF16, tag="bvb", bufs=2, name="bvb_sb")
                nc.sync.dma_start(out=bvb, in_=bvb_d[l][:, :])
                return wq, wk, wv, bvb

            def emit_kproj_group(l, wk, kT, c, j, tag="mm", tagbufs=3,
                                 cast_on_scalar=False):
                base = 48 * l
                kps = ps.tile([128, 512], F32, tag=tag, bufs=tagbufs, name="k_ps")
                for a in range(DT):
                    nc.tensor.matmul(kps, wk[:, a, ds(j * 128, 128)],
                                     memT[:, a, ds(c * 512, 512)],
                                     start=(a == 0), stop=(a == DT - 1))
                dst = kT[:, j, ds(c * 512, 512)]
                bias_ap = prm[:, base + P_BK + j:base + P_BK + j + 1]
                if cast_on_scalar:
                    # keep filler casts off the DVE (busy with the LN chain)
                    nc.scalar.activation(out=dst, in_=kps, func=AF.Identity,
                                         bias=bias_ap, scale=1.0)
                else:
                    nc.vector.tensor_scalar(out=dst, in0=kps, scalar1=bias_ap,
                                            scalar2=None, op0=ALU.add)

            layer_w = [load_layer_weights(0)]
            # filler queue: pending kproj groups of the *next* layer, drained
            # into the PE-idle zones (LN chains / FFN) of the current layer
            filler: list = []

            def drain_filler(n):
                for _ in range(min(n, len(filler))):
                    filler.pop(0)()

            for l in range(L):
                base = 48 * l
                wq, wk, wv, bvb = layer_w[l]

                # --- q projection ----------------------------------------------
                qbf = sb.tile([128, DT, R], BF16, tag="qbf", bufs=1, name="q_sb")
                for j in range(DT):
                    qps = ps.tile([128, R], F32, tag="mm", bufs=3, name="q_ps")
                    for a in range(DT):
                        nc.tensor.matmul(qps, wq[:, a, ds(j * 128, 128)], xbf[:, a, :],
                                         start=(a == 0), stop=(a == DT - 1))
                    nc.vector.tensor_scalar(out=qbf[:, j, :], in0=qps,
                                            scalar1=prm[:, base + P_BQ + j:base + P_BQ + j + 1],
                                            scalar2=None, op0=ALU.add)

                # --- kT projection ([dout, m] in bf16, resident) ----------------
                if l == 0:
                    kT = sb.tile([128, DT, M], BF16, tag="kT", name="kT_sb")
                    for c in range(NCHUNK):
                        for j in range(DT):
                            emit_kproj_group(0, wk, kT, c, j)
                else:
                    kT = kT_next  # noqa: F821  (built by layer l-1's filler)
                    drain_filler(len(filler))  # any leftovers

                # --- attention --------------------------------------------------
                attnps = []
                for j in range(DT):
                    apj = ps.tile([128, R], F32, tag=f"attn{j}", bufs=1, name=f"attn_ps{j}")
                    attnps.append(apj)
                zps = ps.tile([128, R], F32, tag="z", bufs=1, name="z_ps")
                for mt in range(MT):
                    # v for this m-subtile: [m(128), dout(512)] += bias via bvb
                    vps = ps.tile([128, D], F32, tag="mm", bufs=3, name="v_ps")
                    for a in range(DT):
                        nc.tensor.matmul(vps, memT[:, a, ds(mt * 128, 128)], wv[:, a, :],
                                         start=(a == 0), stop=(a == DT - 1))
                    vsb = sb.tile([128, D], BF16, tag="vsb", bufs=4, name="v_sb")
                    nc.vector.tensor_add(vsb, vps, bvb)
                    # scores sT[m(128), r(512)]
                    sps = ps.tile([128, R], F32, tag="mm", bufs=3, name="s_ps")
                    for a in range(DT):
                        nc.tensor.matmul(sps, kT[:, a, ds(mt * 128, 128)], qbf[:, a, :],
                                         start=(a == 0), stop=(a == DT - 1))
                    eraw = sb.tile([128, R], BF16, tag="eraw", bufs=2, name="eraw_sb")
                    nc.scalar.activation(out=eraw, in_=sps, func=AF.Exp)
                    e = sb.tile([128, R], BF16, tag="e", bufs=4, name="e_sb")
                    nc.vector.scalar_tensor_tensor(out=e, in0=bb,
                                                   scalar=iv[:, mt:mt + 1], in1=eraw,
                                                   op0=ALU.is_gt, op1=ALU.mult)
                    nc.tensor.matmul(zps, onesb, e, start=(mt == 0), stop=(mt == MT - 1),
                                     skip_group_check=True)
                    for j in range(DT):
                        nc.tensor.matmul(attnps[j], vsb[:, ds(j * 128, 128)], e,
                                         start=(mt == 0), stop=(mt == MT - 1),
                                         skip_group_check=True)

                # enqueue next layer's kT projection as PE filler work for the
                # LN/FFN zones below (it depends only on memT + next weights)
                if l + 1 < L:
                    layer_w.append(load_layer_weights(l + 1))
                    kT_next = sb.tile([128, DT, M], BF16, tag="kT", name="kTn_sb")
                    wk_next = layer_w[l + 1][1]
                    for c in range(NCHUNK):
                        for j in range(DT):
                            # alternate psum tags so filler groups pipeline
                            # (z bank is idle outside the attention loop)
                            tg, tb = ("z", 1) if (c * DT + j) % 2 else ("mm", 3)
                            filler.append(
                                lambda ll=l + 1, c=c, j=j, wkn=wk_next,
                                ktn=kT_next, tg=tg, tb=tb:
                                emit_kproj_group(ll, wkn, ktn, c, j, tg, tb,
                                                 cast_on_scalar=True))

                # normalize + residual into x
                zt = sb.tile([128, R], F32, tag="at", bufs=2, name="zt_sb")
                nc.scalar.activation(out=zt, in_=zps, func=AF.Copy, bias=1e-9)
                rz = sb.tile([128, R], F32, tag="rz", bufs=1, name="rz_sb")
                nc.vector.reciprocal(rz, zt)
                for j in range(DT):
                    at = sb.tile([128, R], F32, tag="at", bufs=2, name="at_sb")
                    nc.vector.tensor_mul(at, attnps[j], rz)
                    nc.vector.tensor_add(x[:, j, :], x[:, j, :], at)
                drain_filler(6)

                # LN1 (in place), bf16 copy for ffn
                xlnbf = sb.tile([128, DT, R], BF16, tag="xbf", bufs=2, name="xlnbf_sb")
                _layernorm(nc, ps, sb, x, prm, base + P_G1, base + P_BE1, xlnbf,
                           onesi, epsln, drain_filler)

                # FFN1 -> FFN2 fused over the 4D dim: h streams through a small
                # pool; FFN2 psums (one per output subtile) accumulate over o
                # and reuse the attn psum banks (freed above).
                f2ps = []
                for j in range(DT):
                    fpj = ps.tile([128, R], F32, tag=f"attn{j}", bufs=1, name=f"f2_ps{j}")
                    f2ps.append(fpj)

                # weights stream in 512KB chunks (4 o-steps each) — few big
                # DMAs instead of 128 small ones (descriptor issue on the sync
                # engine costs ~600ns per dma_start and serializes)
                w1c, w2c = {}, {}

                def load_ffn_chunk(og):
                    w1c[og] = sb.tile([128, DT, 512], BF16, tag="wc", bufs=4,
                                      name="w1c_sb")
                    nc.sync.dma_start(
                        out=w1c[og],
                        in_=w1_d[l][:, ds(og * 512, 512)].rearrange(
                            "(a p) n -> p a n", p=128))
                    w2c[og] = sb.tile([128, DT, 512], BF16, tag="wc", bufs=4,
                                      name="w2c_sb")
                    nc.sync.dma_start(
                        out=w2c[og],
                        in_=w2_d[l][ds(og * 512, 512), :].rearrange(
                            "(s p) n -> p s n", p=128))

                load_ffn_chunk(0)

                def emit_f2(h, o):
                    for j in range(DT):
                        nc.tensor.matmul(f2ps[j],
                                         w2c[o // 4][:, o % 4, ds(j * 128, 128)], h,
                                         start=(o == 0), stop=(o == FT - 1),
                                         skip_group_check=True)

                # software-pipelined by 2: f2ps matmuls for step o are emitted
                # after FFN1 matmuls for step o+2, hiding the gelu latency from
                # the in-order PE queue
                hq = []
                for o in range(FT):
                    fps = ps.tile([128, R], F32, tag="mm", bufs=3, name="f1_ps")
                    for a in range(DT):
                        nc.tensor.matmul(fps, w1c[o // 4][:, a, ds((o % 4) * 128, 128)],
                                         xlnbf[:, a, :],
                                         start=(a == 0), stop=(a == DT - 1))
                    h = sb.tile([128, R], BF16, tag="h", bufs=4, name="h_sb")
                    nc.scalar.activation(out=h, in_=fps, func=AF.Gelu,
                                         bias=prm[:, base + P_B1 + o:base + P_B1 + o + 1],
                                         scale=1.0)
                    hq.append((h, o))
                    if len(hq) > 2:
                        emit_f2(*hq.pop(0))
                    drain_filler(1)
                    if o % 4 == 3 and o // 4 + 1 < 4:
                        load_ffn_chunk(o // 4 + 1)
                for h_o in hq:
                    emit_f2(*h_o)
                for j in range(DT):
                    nc.vector.scalar_tensor_tensor(out=x[:, j, :], in0=f2ps[j],
                                                   scalar=prm[:, base + P_B2 + j:base + P_B2 + j + 1],
                                                   in1=x[:, j, :],
                                                   op0=ALU.add, op1=ALU.add)

                # pre-start the gate's x0 half on the freed attn psum banks so
                # those matmuls run under the final LN's serial chain
                if l == L - 1:
                    ws0c = sb.tile([128, DT, 512], BF16, tag="wc", bufs=4,
                                   name="ws0c_sb")
                    nc.sync.dma_start(
                        out=ws0c,
                        in_=ws_d[0:512, :].rearrange("(s p) n -> p s n", p=128))
                    ws1c = sb.tile([128, DT, 512], BF16, tag="wc", bufs=4,
                                   name="ws1c_sb")
                    nc.sync.dma_start(
                        out=ws1c,
                        in_=ws_d[512:1024, :].rearrange("(s p) n -> p s n", p=128))
                    gps_l = []
                    for j in range(DT):
                        gp = ps.tile([128, R], F32, tag=f"attn{j}", bufs=1,
                                     name=f"g_ps{j}")
                        for c in range(DT):
                            nc.tensor.matmul(gp, ws0c[:, c, ds(j * 128, 128)],
                                             x0bf[:, c, :],
                                             start=(c == 0), stop=False,
                                             skip_group_check=True)
                        gps_l.append(gp)

                # LN2 (in place) + bf16 copy for next layer / gate
                xbf = sb.tile([128, DT, R], BF16, tag="xbf", bufs=2, name="xbf_sb")
                _layernorm(nc, ps, sb, x, prm, base + P_G2, base + P_BE2, xbf,
                           onesi, epsln, drain_filler)

            # --- item-0 blend: ao = x + flag0*(x0 - x) -------------------------
            ao = sb.tile([128, DT, R], F32, tag="ao", name="ao_sb")
            aobf = sb.tile([128, DT, R], BF16, tag="aobf", name="aobf_sb")
            for j in range(DT):
                dt_ = sb.tile([128, R], F32, tag="gt", bufs=4, name="dt_sb")
                nc.vector.tensor_sub(dt_, x0[:, j, :], x[:, j, :])
                fd = sb.tile([128, R], F32, tag="gt", bufs=4, name="fd_sb")
                nc.vector.tensor_mul(fd, fl, dt_)
                nc.vector.tensor_add(ao[:, j, :], x[:, j, :], fd)
                nc.vector.tensor_copy(out=aobf[:, j, :], in_=ao[:, j, :])

            # --- gate: g = sigmoid(ws.T @ [x0; ao] + bs) -----------------------
            # (x0 half already accumulated in gps_l; finish with the ao half)
            for j in range(DT):
                gps = gps_l[j]
                for c in range(DT):
                    nc.tensor.matmul(gps, ws1c[:, c, ds(j * 128, 128)],
                                     aobf[:, c, :],
                                     start=False, stop=(c == DT - 1),
                                     skip_group_check=True)
                g = sb.tile([128, R], F32, tag="gt", bufs=4, name="g_sb")
                nc.scalar.activation(out=g, in_=gps, func=AF.Sigmoid,
                                     bias=prm[:, P_BS + j:P_BS + j + 1], scale=1.0)
                # out = ao + g*(x0 - ao)
                d2 = sb.tile([128, R], F32, tag="gt", bufs=4, name="d2_sb")
                nc.vector.tensor_sub(d2, x0[:, j, :], ao[:, j, :])
                m2 = sb.tile([128, R], F32, tag="gt", bufs=4, name="m2_sb")
                nc.vector.tensor_mul(m2, g, d2)
                ov = sb.tile([128, R], F32, tag="gt", bufs=4, name="ov_sb")
                nc.vector.tensor_add(ov, ao[:, j, :], m2)
                nc.sync.dma_start(out=out_d[j * 128:(j + 1) * 128, :], in_=ov)

    nc.compile()
    return nc


_NC = None


def _get_nc():
    global _NC
    if _NC is None:
        _NC = _build()
    return _NC


def _make_in_maps(inputs):
    cog = np.asarray(inputs["cognition_features"], np.float32)
    flat = cog.reshape(M, D)
    cogT = np.ascontiguousarray(flat.T)          # [D, M] fp32
    memT_bf = cogT.astype(BF)
    sc = 1.0 / np.sqrt(np.float32(D))

    common = {"memT": memT_bf}
    for l in range(L):
        common[f"wq{l}"] = np.ascontiguousarray(np.asarray(inputs["Wq"][l], np.float32).T * sc).astype(BF)
        common[f"wk{l}"] = np.ascontiguousarray(np.asarray(inputs["Wk"][l], np.float32).T).astype(BF)
        common[f"wv{l}"] = np.ascontiguousarray(np.asarray(inputs["Wv"][l], np.float32).T).astype(BF)
        common[f"w1{l}"] = np.ascontiguousarray(np.asarray(inputs["W1"][l], np.float32).T).astype(BF)
        common[f"w2{l}"] = np.ascontiguousarray(np.asarray(inputs["W2"][l], np.float32).T).astype(BF)
        common[f"bvb{l}"] = np.broadcast_to(
            np.asarray(inputs["bv"][l], np.float32), (128, D)).astype(BF).copy()
    common["ws"] = np.ascontiguousarray(np.asarray(inputs["Ws"], np.float32).T).astype(BF)

    prm = np.zeros((128, P_COLS), np.float32)

    def put(col, vec):
        v = np.asarray(vec, np.float32).reshape(-1, 128)
        for j in range(v.shape[0]):
            prm[:, col + j] = v[j]

    for l in range(L):
        base = 48 * l
        put(base + P_BQ, np.asarray(inputs["bq"][l], np.float32) * sc)
        put(base + P_BK, inputs["bk"][l])
        put(base + P_G1, inputs["ln1_g"][l])
        put(base + P_BE1, inputs["ln1_b"][l])
        put(base + P_B1, inputs["b1"][l])
        put(base + P_B2, inputs["b2"][l])
        put(base + P_G2, inputs["ln2_g"][l])
        put(base + P_BE2, inputs["ln2_b"][l])
    put(P_BS, inputs["bs"])
    common["params"] = prm

    item_of_m = (np.arange(M) // T).astype(np.float32)
    iv = np.ascontiguousarray(item_of_m.reshape(MT, 128).T).astype(BF)
    common["item_vals"] = iv

    in_maps = []
    for d in range(NCORES):
        rows = slice(d * R, (d + 1) * R)
        b_of_r = (np.arange(d * R, (d + 1) * R) // T).astype(np.float32)
        im = dict(common)
        im["xT0"] = np.ascontiguousarray(cogT[:, rows])
        im["xT0bf"] = np.ascontiguousarray(cogT[:, rows]).astype(BF)
        im["b_bcast"] = np.broadcast_to(b_of_r, (128, R)).astype(BF).copy()
        im["flag0"] = np.broadcast_to((b_of_r == 0).astype(np.float32), (128, R)).copy()
        in_maps.append(im)
    return in_maps


def _run(in_maps, trace=False):
    nc = _get_nc()
    return run_bass_kernel_spmd(nc, in_maps, list(range(NCORES)), trace=trace)


def kernel(**inputs):
    in_maps = _make_in_maps(inputs)
    res = _run(in_maps)
    outT = np.empty((M, D), np.float32)
    for d in range(NCORES):
        outT[d * R:(d + 1) * R, :] = res.results[d]["outT"].T
    return outT.reshape(B, T, D)


if __name__ == "__main__":
    _build()
    print("build ok")
